# revision 15
# baseline (speedup 1.0000x reference)
"""Trainium2 Bass kernel for block-local (chunked) attention.

Problem: x:(4,4096,1024) f32. qkv = x @ w_qkv.T; block-local attention with
chunk=64 inside each head (16 heads, dim_head 64); out proj w_out + b_out.

Strategy (8 cores, SPMD):
  - Shard the 16384 flattened tokens into 8 contiguous shards of 2048
    (chunk-aligned, so blocks never cross shards).
  - Host pre-transposes x and the weights so every DMA is contiguous and
    every matmul operand has the contraction dim on partitions.
  - Per core: qkv projection (bf16 matmuls, fp32 PSUM accumulate), block
    attention with the softmax reduction done ON the partition axis via a
    block-identity matmul (sum + broadcast in one PE op), final projection,
    pipelined with the Tile framework.

Perf notes (measured on HW):
  - PE streams 1 moving column/cycle at ~2.0 GHz sustained (N=512 MM paces
    at 257 ns); the kernel is stream-rate bound, so the wins are: no idle
    at startup/tail and sub-array concurrency for the half-array
    attention matmuls.
  - Each DMA_DIRECT2D costs ~0.7 us of Sync-engine issue time and DMA
    completion latency is ~5 us, so DMAs are BATCHED: 2 per x group
    (kc0 first so the first chain starts early), 4 for q/k weights,
    2 for v weights, 1 merged y store per group (was 8).
  - Scores (K=64, stationary rows 0-63 vs 64-127) and attn@v (M=64, out
    cols 0-63 vs 64-127) are emitted parity-interleaved so adjacent MMs
    occupy disjoint PE sub-arrays and run concurrently.
  - v projection runs kc-outer with both feature-half matmuls per x-block
    so the stationary loads amortize.
  - y returned as bf16 (halves output DMA); b_out added on host in f32.

Layouts on device (P=128 partitions always first):
  xT     [128, 8, T]      bf16   xT[p,kc,t]       = x_shard[t, kc*128+p]
  wqkvT  [128, 24, 1024]  bf16   [p,fc,kc*128+fr] = w_qkv[fc*128+fr, kc*128+p]
  woutT  [128, 8, 1024]   bf16   [p,hp,e]         = w_out[e, hp*128+p]
  consts [128, 128]       bf16   [:, :64]=upper-half ones, [:, 64:]=lower
  yT     [128, 8, T]      bf16   yT[p,ec,t]       = y_shard[t, ec*128+p]

HW gotcha baked in below: matmuls whose stationary operands live at
different base partitions (row groups 0 vs 64) must never target the same
PSUM bank — that crashes the device. Scores matmuls are therefore grouped
by head parity into separate PSUM tiles (sc_a / sc_b).
"""

import os
import sys

for _p in ("/opt/trn_rl_repo", "/root/.axon_site/_ro/trn_rl_repo"):
    if os.path.isdir(_p) and _p not in sys.path:
        sys.path.append(_p)

import numpy as np
import ml_dtypes

import concourse.bass as bass
from concourse import bacc
from concourse import mybir
from concourse import tile

BF16 = mybir.dt.bfloat16
F32 = mybir.dt.float32
BF16_NP = ml_dtypes.bfloat16

P = 128
KC = 8            # contraction chunks for dim=1024
HEADS = 16
DH = 64
CHUNK = 64
INNER = HEADS * DH            # 1024
DIM = 1024
N_CORES = 8
ST = 128                      # tokens per attention subtile
NFC = 24                      # feature chunks of 128 in wqkv (q8, k8, v8)
SCALE = DH ** -0.5
GT = 512                      # tokens per group
NST = GT // ST

# Scores/attn@v emission order: alternate head parity so adjacent matmuls
# use disjoint PE sub-arrays (rows 0-63 vs 64-127 for scores; out cols
# 0-63 vs 64-127 for attn@v) and overlap in the array.
JORDER = [0, 4, 1, 5, 2, 6, 3, 7]


def build_body(tc, yT, xT, wqkvT, woutT, consts, T):
    """Emit the whole per-core program into TileContext tc."""
    nc = tc.nc
    G = T // GT
    import contextlib
    ctx = contextlib.ExitStack()

    # --- SBUF pools -----------------------------------------------------
    wpool = ctx.enter_context(tc.tile_pool(name="w", bufs=1))
    xpool = ctx.enter_context(tc.tile_pool(name="x", bufs=2))
    qkpool = ctx.enter_context(tc.tile_pool(name="qk", bufs=2))
    vpool = ctx.enter_context(tc.tile_pool(name="v", bufs=3))
    epool = ctx.enter_context(tc.tile_pool(name="e", bufs=2))
    rbpool = ctx.enter_context(tc.tile_pool(name="rb", bufs=2))
    enpool = ctx.enter_context(tc.tile_pool(name="en", bufs=1))
    apool = ctx.enter_context(tc.tile_pool(name="a", bufs=2))
    ypool = ctx.enter_context(tc.tile_pool(name="y", bufs=3))

    # --- PSUM pools: 2 + 2 + 2 + 1 + 1 = 8 banks ------------------------
    ps_big = ctx.enter_context(tc.tile_pool(name="ps_big", bufs=2, space="PSUM"))
    ps_v = ctx.enter_context(tc.tile_pool(name="ps_v", bufs=2, space="PSUM"))
    ps_sc = ctx.enter_context(tc.tile_pool(name="ps_sc", bufs=2, space="PSUM"))
    ps_sm = ctx.enter_context(tc.tile_pool(name="ps_sm", bufs=1, space="PSUM"))
    ps_at = ctx.enter_context(tc.tile_pool(name="ps_at", bufs=1, space="PSUM"))

    # --- startup DMAs, batched, in arrival-priority order ---------------
    # First x chunk (kc=0) + first q/k weight slab gate the first matmul;
    # everything else streams in behind them.
    def load_x(dst, t0):
        nc.sync.dma_start(dst[:, 0:GT], xT[:, 0, t0:t0 + GT])
        nc.sync.dma_start(
            dst[:, GT:KC * GT].rearrange("p (k t) -> p k t", k=KC - 1),
            xT[:, 1:KC, t0:t0 + GT],
        )

    # Order matters: the single DMA queue drains in issue order, so the
    # two blobs gating the first matmul chain go first (x chunk kc=0,
    # 128 KiB + first q/k weight slab, 1 MiB), then the rest of x, then
    # the remaining weights.
    x_first = xpool.tile([P, KC * GT], BF16, tag="x", name="x_first")
    wqk_sb = wpool.tile([P, 16 * 1024], BF16, tag="wqk")

    def load_wqk(f0, f1):
        nc.sync.dma_start(
            wqk_sb[:, f0 * 1024:f1 * 1024].rearrange(
                "p (f c) -> p f c", f=f1 - f0),
            wqkvT[:, f0:f1, :],
        )

    nc.sync.dma_start(x_first[:, 0:GT], xT[:, 0, 0:GT])
    load_wqk(0, 2)
    nc.sync.dma_start(
        x_first[:, GT:4 * GT].rearrange("p (k t) -> p k t", k=3),
        xT[:, 1:4, 0:GT],
    )
    load_wqk(2, 4)
    nc.sync.dma_start(
        x_first[:, 4 * GT:KC * GT].rearrange("p (k t) -> p k t", k=KC - 4),
        xT[:, 4:KC, 0:GT],
    )
    load_wqk(4, 8)
    load_wqk(8, 12)
    load_wqk(12, 16)
    wv_sb = wpool.tile([P, 8 * 1024], BF16, tag="wv")
    for i in range(2):
        nc.sync.dma_start(
            wv_sb[:, i * 4096:(i + 1) * 4096].rearrange(
                "p (f c) -> p f c", f=4),
            wqkvT[:, 16 + 4 * i:20 + 4 * i, :],
        )
    wv3 = wv_sb[:].rearrange("p (f c) -> p f c", f=8)
    consts_sb = wpool.tile([P, 128], BF16, tag="consts")
    nc.sync.dma_start(consts_sb[:], consts[:, :])
    wout_sb = wpool.tile([P, KC * 1024], BF16, tag="wout")
    wout_loaded = [False]

    def load_wout():
        if not wout_loaded[0]:
            nc.sync.dma_start(
                wout_sb[:].rearrange("p (k e) -> p k e", k=KC), woutT[:, :, :]
            )
            wout_loaded[0] = True

    # --- persistent pre-zeroed normalized-probs tiles -------------------
    # En garbage regions (cross-chunk blocks) stay zero forever; only the
    # valid block-diagonal regions are rewritten each iteration, letting the
    # attn@v matmul contract over the full 128 partitions in one shot.
    en_tiles = []
    for i in range(4):
        t = enpool.tile([P, 8 * 128], BF16, tag=f"en{i}", name=f"en{i}")
        nc.gpsimd.memset(t[:], 0.0)
        en_tiles.append(t)

    half_idx = [0]
    x_tiles = [x_first]

    # ---- v projection, split in two halves so it can fill the PE queue
    # between attention matmuls that wait on the scalar/vector softmax
    # chain (the tensor queue is strict FIFO — a waiting matmul blocks
    # everything behind it, so the filler must be emitted in between).
    def make_v(g, st):
        state = {}

        def fill(lo, hi):
            if lo == 0:
                state["v_sb"] = vpool.tile([P, INNER], BF16, tag="v",
                                           name=f"v_{g}_{st}")
                state["ps"] = (ps_v.tile([P, 512], F32, tag="v", name="v_ps0"),
                               ps_v.tile([P, 512], F32, tag="v", name="v_ps1"))
            x_t = x_tiles[g]
            vps = state["ps"]
            for kc in range(lo, hi):
                lhs = x_t[:, kc * GT + st * ST: kc * GT + st * ST + ST]
                for half in range(2):
                    nc.tensor.matmul(
                        vps[half][:],
                        lhsT=lhs,
                        rhs=wv3[:, 4 * half:4 * half + 4,
                                kc * 128:(kc + 1) * 128],
                        start=(kc == 0),
                        stop=(kc == KC - 1),
                    )
            if hi == KC:
                v_sb = state["v_sb"]
                for half in range(2):
                    nc.vector.tensor_copy(
                        v_sb[:, half * 512:(half + 1) * 512], vps[half][:]
                    )
                return v_sb
            return None

        return fill

    def scores_half(q_sb, k_sb, st, hh):
        # scoresT[tk, tq] per head: stationary kT, moving qT.
        # E column block j holds head h = hh*8 + 2*(j%4) + (j//4): even
        # heads (stationary base partition 0) fill sc_a, odd heads (base
        # 64) fill sc_b — mixing row groups within one PSUM bank is fatal
        # on TRN2 hardware. JORDER alternates parity so adjacent matmuls
        # overlap in the PE array.
        sc_a = ps_sc.tile([P, 512], F32, tag="sc")
        sc_b = ps_sc.tile([P, 512], F32, tag="sc")
        for j in JORDER:
            h = hh * 8 + 2 * (j % 4) + (j // 4)
            fc = h // 2
            hb = (h % 2) * 64
            sc = sc_a if j < 4 else sc_b
            nc.tensor.matmul(
                sc[:, (j % 4) * 128:(j % 4 + 1) * 128],
                lhsT=k_sb[hb:hb + 64, fc * GT + st * ST: fc * GT + st * ST + ST],
                rhs=q_sb[hb:hb + 64, fc * GT + st * ST: fc * GT + st * ST + ST],
                start=True,
                stop=True,
            )
        # exp (scale folded in); no max-subtraction needed: scores~N(0,1)
        e_sb = epool.tile([P, 8 * 128], BF16, tag="e")
        nc.scalar.activation(
            e_sb[:, 0:512], sc_a[:],
            mybir.ActivationFunctionType.Exp, scale=SCALE,
        )
        nc.scalar.activation(
            e_sb[:, 512:1024], sc_b[:],
            mybir.ActivationFunctionType.Exp, scale=SCALE,
        )
        return e_sb

    def denom_half(e_sb):
        # denominators: block-identity matmul sums over tk (partition
        # axis) AND broadcasts the result to all 64 partitions of the
        # matching chunk. Garbage cross-chunk scores are excluded by the
        # zeros in the stationary.
        e3 = e_sb[:].rearrange("p (h q) -> p h q", h=8)
        sm_ps = ps_sm.tile([P, 512], F32, tag="sm")
        nc.tensor.matmul(
            sm_ps[0:64, :],
            lhsT=consts_sb[:, 0:64],
            rhs=e3[:, :, 0:64],
            start=True, stop=True,
        )
        nc.tensor.matmul(
            sm_ps[64:128, :],
            lhsT=consts_sb[:, 64:128],
            rhs=e3[:, :, 64:128],
            start=True, stop=True,
        )
        rb = rbpool.tile([P, 512], F32, tag="rb")
        nc.vector.reciprocal_approx_fast(out=rb[:], in_=sm_ps[:])
        # normalize: En = E * rb  (valid block-diagonal regions only)
        en = en_tiles[half_idx[0] % 4]
        half_idx[0] += 1
        en3 = en[:].rearrange("p (h q) -> p h q", h=8)
        rb3 = rb[:].rearrange("p (h q) -> p h q", h=8)
        nc.vector.tensor_mul(
            en3[0:64, :, 0:64], e3[0:64, :, 0:64], rb3[0:64, :, :]
        )
        nc.vector.tensor_mul(
            en3[64:128, :, 64:128], e3[64:128, :, 64:128], rb3[64:128, :, :]
        )
        return en

    def attnv_half(v_sb, en, attn_sb, st, hh, at_pool, at_tag):
        # attn @ v : outT[d(head), tok] — full-partition contract, En's
        # zeros kill the cross-chunk terms. JORDER alternates the output
        # column group (hb) for sub-array overlap.
        at_ps = at_pool.tile([P, 512], F32, tag=at_tag, name="at_ps")
        for j in JORDER:
            h = hh * 8 + 2 * (j % 4) + (j // 4)
            hb = (h % 2) * 64
            nc.tensor.matmul(
                at_ps[hb:hb + 64, (j % 4) * 128:(j % 4 + 1) * 128],
                lhsT=v_sb[:, h * 64:(h + 1) * 64],
                rhs=en[:, j * 128:(j + 1) * 128],
                start=True, stop=True,
            )
        nc.scalar.copy(
            attn_sb[:, st * 1024 + hh * 512: st * 1024 + (hh + 1) * 512],
            at_ps[:],
        )

    for g in range(G):
        x_t = x_tiles[g]

        # ---- q/k projections: out layout [feat, tok] -------------------
        q_sb = qkpool.tile([P, 8 * GT], BF16, tag="q")
        k_sb = qkpool.tile([P, 8 * GT], BF16, tag="k")
        attn_sb = apool.tile([P, NST * 8 * 128], BF16, tag="attn",
                             name=f"attn_{g}")
        for fc in range(16):
            qk_ps = ps_big.tile([P, GT], F32, tag="big")
            for kc in range(KC):
                nc.tensor.matmul(
                    qk_ps[:],
                    lhsT=wqk_sb[:, fc * 1024 + kc * 128: fc * 1024 + kc * 128 + 128],
                    rhs=x_t[:, kc * GT:(kc + 1) * GT],
                    start=(kc == 0),
                    stop=(kc == KC - 1),
                )
            dst = q_sb if fc < 8 else k_sb
            sl = dst[:, (fc % 8) * GT:(fc % 8 + 1) * GT]
            if fc % 2 == 0:
                nc.vector.tensor_copy(sl, qk_ps[:])
            else:
                nc.scalar.copy(sl, qk_ps[:])

        # prefetch next group's x now, before this group's y store can
        # block the Sync queue
        if g + 1 < G:
            x_n = xpool.tile([P, KC * GT], BF16, tag="x")
            load_x(x_n, (g + 1) * GT)
            x_tiles.append(x_n)
        if g == 0:
            load_wout()

        # ---- per 128-token subtile: v projection + attention -----------
        for st in range(NST):
            fill = make_v(g, st)
            fill(0, 4)
            v_sb = fill(4, KC)
            e0 = scores_half(q_sb, k_sb, st, 0)
            en0 = denom_half(e0)
            attnv_half(v_sb, en0, attn_sb, st, 0, ps_at, "at")
            e1 = scores_half(q_sb, k_sb, st, 1)
            en1 = denom_half(e1)
            attnv_half(v_sb, en1, attn_sb, st, 1, ps_at, "at")

        # ---- final projection ------------------------------------------
        # For the last group there is no following work to hide the
        # attention→final serialization, so split it into two token halves:
        # the first half only needs subtiles 0..NST/2-1 and overlaps the
        # rest of the attention. y is packed ec-major into one tile so the
        # store is a single DMA.
        a3 = attn_sb[:].rearrange("p (s h t) -> p s h t", s=NST, h=8)
        halves = ((0, NST // 2), (NST // 2, NST)) if g == G - 1 \
            else ((0, NST),)
        for s0, s1 in halves:
            ht = (s1 - s0) * ST
            y_sb = ypool.tile([P, 8 * GT], BF16, tag="y")
            final = g == G - 1 and s0 > 0
            for ec in range(8):
                f_ps = ps_big.tile([P, GT], F32, tag="big")
                for hp in range(KC):
                    nc.tensor.matmul(
                        f_ps[:, 0:ht],
                        lhsT=wout_sb[:, hp * 1024 + ec * 128: hp * 1024 + ec * 128 + 128],
                        rhs=a3[:, s0:s1, hp, :],
                        start=(hp == 0),
                        stop=(hp == KC - 1),
                    )
                sl = y_sb[:, ec * ht:(ec + 1) * ht]
                if ec % 2 == 0:
                    nc.vector.tensor_copy(sl, f_ps[:, 0:ht])
                else:
                    nc.scalar.copy(sl, f_ps[:, 0:ht])
                if final and ec % 2 == 1:
                    # final store: stream out per ec-pair as soon as the
                    # copies land, so only the last 128 KiB pays the DMA
                    # completion latency after the last matmul
                    nc.sync.dma_start(
                        yT[:, ec - 1:ec + 1, g * GT + s0 * ST: g * GT + s1 * ST],
                        y_sb[:, (ec - 1) * ht:(ec + 1) * ht].rearrange(
                            "p (e t) -> p e t", e=2),
                    )
            if not final:
                nc.sync.dma_start(
                    yT[:, :, g * GT + s0 * ST: g * GT + s1 * ST],
                    y_sb[:, 0:8 * ht].rearrange("p (e t) -> p e t", e=8),
                )

    ctx.close()


def build_nc(T):
    nc = bacc.Bacc("TRN2", target_bir_lowering=False, debug=False)
    xT = nc.dram_tensor("xT", [P, KC, T], BF16, kind="ExternalInput").ap()
    wqkvT = nc.dram_tensor("wqkvT", [P, NFC, 1024], BF16, kind="ExternalInput").ap()
    woutT = nc.dram_tensor("woutT", [P, KC, 1024], BF16, kind="ExternalInput").ap()
    consts = nc.dram_tensor("consts", [P, 128], BF16, kind="ExternalInput").ap()
    yT = nc.dram_tensor("yT", [P, KC, T], BF16, kind="ExternalOutput").ap()
    with tile.TileContext(nc) as tc:
        build_body(tc, yT, xT, wqkvT, woutT, consts, T)
    nc.compile()
    return nc


def make_consts():
    c = np.zeros((P, 128), dtype=BF16_NP)
    c[0:64, 0:64] = 1
    c[64:128, 64:128] = 1
    return c


def prep_inputs(x, w_qkv, w_out, T):
    """Host-side shard + transpose + cast. Returns in_maps list for SPMD."""
    tok = x.shape[0] * x.shape[1]
    flat = np.ascontiguousarray(x.reshape(tok, DIM))
    # [fc, fr, kc, p] -> [p, fc, kc, fr] -> [p, fc, kc*128+fr]
    wqkvT = np.ascontiguousarray(
        w_qkv.reshape(NFC, 128, KC, 128).transpose(3, 0, 2, 1).reshape(P, NFC, 1024)
    ).astype(BF16_NP)
    woutT = np.ascontiguousarray(
        w_out.T.reshape(KC, P, 1024).transpose(1, 0, 2)
    ).astype(BF16_NP)
    consts = make_consts()
    n_cores = tok // T
    in_maps = []
    for c in range(n_cores):
        shard = flat[c * T:(c + 1) * T]           # [T, 1024]
        xTc = np.ascontiguousarray(
            shard.T.reshape(KC, P, T).transpose(1, 0, 2)
        ).astype(BF16_NP)
        in_maps.append({"xT": xTc, "wqkvT": wqkvT, "woutT": woutT,
                        "consts": consts})
    return in_maps


def postprocess(results, b_out, bshape, T):
    outs = []
    for r in results:
        yT = np.asarray(r["yT"], dtype=np.float32)    # [128, 8, T] (bf16)
        outs.append(yT.transpose(2, 1, 0).reshape(T, DIM))   # [T, 1024]
    y = np.concatenate(outs, axis=0)                  # [tok, 1024]
    y = y + np.asarray(b_out, dtype=np.float32)[None, :]
    return y.reshape(*bshape, DIM)


_CACHED = {}


def kernel(x, w_qkv, w_out, b_out):
    from concourse.bass_utils import run_bass_kernel_spmd

    x = np.asarray(x)
    b, n, _ = x.shape
    T = (b * n) // N_CORES
    if T not in _CACHED:
        _CACHED[T] = build_nc(T)
    nc = _CACHED[T]
    in_maps = prep_inputs(x, np.asarray(w_qkv), np.asarray(w_out), T)
    res = run_bass_kernel_spmd(nc, in_maps, list(range(N_CORES)))
    return postprocess(res.results, b_out, (b, n), T)


if __name__ == "__main__":
    nc = build_nc(2048)
    print("built ok")


# revision 17
# speedup vs baseline: 1.1861x; 1.1861x over previous
"""Trainium2 Bass kernel for block-local (chunked) attention.

Problem: x:(4,4096,1024) f32. qkv = x @ w_qkv.T; block-local attention with
chunk=64 inside each head (16 heads, dim_head 64); out proj w_out + b_out.

Strategy (8 cores, SPMD):
  - Shard the 16384 flattened tokens into 8 contiguous shards of 2048
    (chunk-aligned, so blocks never cross shards).
  - Host pre-transposes x and the weights so every DMA is contiguous and
    every matmul operand has the contraction dim on partitions.
  - Per core: qkv projection (bf16 matmuls, fp32 PSUM accumulate), block
    attention with the softmax reduction done ON the partition axis via a
    block-identity matmul (sum + broadcast in one PE op), final projection,
    pipelined with the Tile framework.

Perf notes (measured on HW):
  - PE streams 1 moving column/cycle at ~2.0 GHz sustained (N=512 MM paces
    at 257 ns); the kernel is stream-rate bound, so the wins are: no idle
    at startup/tail and sub-array concurrency for the half-array
    attention matmuls.
  - Each DMA_DIRECT2D costs ~0.7 us of Sync-engine issue time and DMA
    completion latency is ~5 us, so DMAs are BATCHED: 2 per x group
    (kc0 first so the first chain starts early), 4 for q/k weights,
    2 for v weights, 1 merged y store per group (was 8).
  - Scores (K=64, stationary rows 0-63 vs 64-127) and attn@v (M=64, out
    cols 0-63 vs 64-127) are emitted parity-interleaved so adjacent MMs
    occupy disjoint PE sub-arrays and run concurrently.
  - v projection runs kc-outer with both feature-half matmuls per x-block
    so the stationary loads amortize.
  - y returned as bf16 (halves output DMA); b_out added on host in f32.

Layouts on device (P=128 partitions always first):
  xT     [128, 8, T]      bf16   xT[p,kc,t]       = x_shard[t, kc*128+p]
  wqkvT  [128, 24, 1024]  bf16   [p,fc,kc*128+fr] = w_qkv[fc*128+fr, kc*128+p]
  woutT  [128, 8, 1024]   bf16   [p,hp,e]         = w_out[e, hp*128+p]
  consts [128, 128]       bf16   [:, :64]=upper-half ones, [:, 64:]=lower
  yT     [128, 8, T]      bf16   yT[p,ec,t]       = y_shard[t, ec*128+p]

HW gotcha baked in below: matmuls whose stationary operands live at
different base partitions (row groups 0 vs 64) must never target the same
PSUM bank — that crashes the device. Scores matmuls are therefore grouped
by head parity into separate PSUM tiles (sc_a / sc_b).
"""

import os
import sys

for _p in ("/opt/trn_rl_repo", "/root/.axon_site/_ro/trn_rl_repo"):
    if os.path.isdir(_p) and _p not in sys.path:
        sys.path.append(_p)

import numpy as np
import ml_dtypes

import concourse.bass as bass
from concourse import bacc
from concourse import mybir
from concourse import tile

BF16 = mybir.dt.bfloat16
F32 = mybir.dt.float32
BF16_NP = ml_dtypes.bfloat16

P = 128
KC = 8            # contraction chunks for dim=1024
HEADS = 16
DH = 64
CHUNK = 64
INNER = HEADS * DH            # 1024
DIM = 1024
N_CORES = 8
ST = 128                      # tokens per attention subtile
NFC = 24                      # feature chunks of 128 in wqkv (q8, k8, v8)
SCALE = DH ** -0.5
GT = 512                      # tokens per group
NST = GT // ST

# Scores/attn@v emission order: alternate head parity so adjacent matmuls
# use disjoint PE sub-arrays (rows 0-63 vs 64-127 for scores; out cols
# 0-63 vs 64-127 for attn@v) and overlap in the array.
JORDER = [0, 4, 1, 5, 2, 6, 3, 7]


def build_body(tc, yT, xT, wqkvT, woutT, consts, T):
    """Emit the whole per-core program into TileContext tc."""
    nc = tc.nc
    G = T // GT
    import contextlib
    ctx = contextlib.ExitStack()

    # --- SBUF pools -----------------------------------------------------
    wpool = ctx.enter_context(tc.tile_pool(name="w", bufs=1))
    xpool = ctx.enter_context(tc.tile_pool(name="x", bufs=2))
    qkpool = ctx.enter_context(tc.tile_pool(name="qk", bufs=2))
    vpool = ctx.enter_context(tc.tile_pool(name="v", bufs=3))
    epool = ctx.enter_context(tc.tile_pool(name="e", bufs=2))
    rbpool = ctx.enter_context(tc.tile_pool(name="rb", bufs=2))
    enpool = ctx.enter_context(tc.tile_pool(name="en", bufs=1))
    apool = ctx.enter_context(tc.tile_pool(name="a", bufs=2))
    ypool = ctx.enter_context(tc.tile_pool(name="y", bufs=3))

    # --- PSUM pools: 2 + 2 + 2 + 1 + 1 = 8 banks ------------------------
    ps_big = ctx.enter_context(tc.tile_pool(name="ps_big", bufs=2, space="PSUM"))
    ps_v = ctx.enter_context(tc.tile_pool(name="ps_v", bufs=2, space="PSUM"))
    ps_sc = ctx.enter_context(tc.tile_pool(name="ps_sc", bufs=2, space="PSUM"))
    ps_sm = ctx.enter_context(tc.tile_pool(name="ps_sm", bufs=1, space="PSUM"))
    ps_at = ctx.enter_context(tc.tile_pool(name="ps_at", bufs=1, space="PSUM"))

    # --- startup DMAs, batched, in arrival-priority order ---------------
    # First x chunk (kc=0) + first q/k weight slab gate the first matmul;
    # everything else streams in behind them.
    def load_x(dst, t0):
        nc.sync.dma_start(dst[:, 0:GT], xT[:, 0, t0:t0 + GT])
        nc.sync.dma_start(
            dst[:, GT:KC * GT].rearrange("p (k t) -> p k t", k=KC - 1),
            xT[:, 1:KC, t0:t0 + GT],
        )

    # Order matters: the single DMA queue drains in issue order, so the
    # two blobs gating the first matmul chain go first (x chunk kc=0,
    # 128 KiB + first q/k weight slab, 1 MiB), then the rest of x, then
    # the remaining weights.
    x_first = xpool.tile([P, KC * GT], BF16, tag="x", name="x_first")
    wqk_sb = wpool.tile([P, 16 * 1024], BF16, tag="wqk")

    def load_wqk(f0, f1):
        nc.sync.dma_start(
            wqk_sb[:, f0 * 1024:f1 * 1024].rearrange(
                "p (f c) -> p f c", f=f1 - f0),
            wqkvT[:, f0:f1, :],
        )

    nc.sync.dma_start(x_first[:, 0:GT], xT[:, 0, 0:GT])
    load_wqk(0, 2)
    nc.sync.dma_start(
        x_first[:, GT:KC * GT].rearrange("p (k t) -> p k t", k=KC - 1),
        xT[:, 1:KC, 0:GT],
    )
    load_wqk(2, 4)
    load_wqk(4, 8)
    load_wqk(8, 12)
    load_wqk(12, 16)
    wv_sb = wpool.tile([P, 8 * 1024], BF16, tag="wv")
    for i in range(2):
        nc.sync.dma_start(
            wv_sb[:, i * 4096:(i + 1) * 4096].rearrange(
                "p (f c) -> p f c", f=4),
            wqkvT[:, 16 + 4 * i:20 + 4 * i, :],
        )
    wv3 = wv_sb[:].rearrange("p (f c) -> p f c", f=8)
    consts_sb = wpool.tile([P, 128], BF16, tag="consts")
    nc.sync.dma_start(consts_sb[:], consts[:, :])
    wout_sb = wpool.tile([P, KC * 1024], BF16, tag="wout")
    wout_loaded = [False]

    def load_wout():
        if not wout_loaded[0]:
            nc.sync.dma_start(
                wout_sb[:].rearrange("p (k e) -> p k e", k=KC), woutT[:, :, :]
            )
            wout_loaded[0] = True

    # --- persistent pre-zeroed normalized-probs tiles -------------------
    # En garbage regions (cross-chunk blocks) stay zero forever; only the
    # valid block-diagonal regions are rewritten each iteration, letting the
    # attn@v matmul contract over the full 128 partitions in one shot.
    en_tiles = []
    for i in range(4):
        t = enpool.tile([P, 8 * 128], BF16, tag=f"en{i}", name=f"en{i}")
        nc.gpsimd.memset(t[:], 0.0)
        en_tiles.append(t)

    half_idx = [0]
    x_tiles = [x_first]

    # ---- v projection, split in two halves so it can fill the PE queue
    # between attention matmuls that wait on the scalar/vector softmax
    # chain (the tensor queue is strict FIFO — a waiting matmul blocks
    # everything behind it, so the filler must be emitted in between).
    def make_v(g, st):
        state = {}

        def fill(lo, hi):
            if lo == 0:
                state["v_sb"] = vpool.tile([P, INNER], BF16, tag="v",
                                           name=f"v_{g}_{st}")
                state["ps"] = (ps_v.tile([P, 512], F32, tag="v", name="v_ps0"),
                               ps_v.tile([P, 512], F32, tag="v", name="v_ps1"))
            x_t = x_tiles[g]
            vps = state["ps"]
            for kc in range(lo, hi):
                lhs = x_t[:, kc * GT + st * ST: kc * GT + st * ST + ST]
                for half in range(2):
                    nc.tensor.matmul(
                        vps[half][:],
                        lhsT=lhs,
                        rhs=wv3[:, 4 * half:4 * half + 4,
                                kc * 128:(kc + 1) * 128],
                        start=(kc == 0),
                        stop=(kc == KC - 1),
                    )
            if hi == KC:
                v_sb = state["v_sb"]
                for half in range(2):
                    nc.vector.tensor_copy(
                        v_sb[:, half * 512:(half + 1) * 512], vps[half][:]
                    )
                return v_sb
            return None

        return fill

    def scores_half(q_sb, k_sb, st, hh):
        # scoresT[tk, tq] per head: stationary kT, moving qT.
        # E column block j holds head h = hh*8 + 2*(j%4) + (j//4): even
        # heads (stationary base partition 0) fill sc_a, odd heads (base
        # 64) fill sc_b — mixing row groups within one PSUM bank is fatal
        # on TRN2 hardware. JORDER alternates parity so adjacent matmuls
        # overlap in the PE array.
        sc_a = ps_sc.tile([P, 512], F32, tag="sc")
        sc_b = ps_sc.tile([P, 512], F32, tag="sc")
        for j in JORDER:
            h = hh * 8 + 2 * (j % 4) + (j // 4)
            fc = h // 2
            hb = (h % 2) * 64
            sc = sc_a if j < 4 else sc_b
            nc.tensor.matmul(
                sc[:, (j % 4) * 128:(j % 4 + 1) * 128],
                lhsT=k_sb[hb:hb + 64, fc * GT + st * ST: fc * GT + st * ST + ST],
                rhs=q_sb[hb:hb + 64, fc * GT + st * ST: fc * GT + st * ST + ST],
                start=True,
                stop=True,
            )
        # exp (scale folded in); no max-subtraction needed: scores~N(0,1)
        e_sb = epool.tile([P, 8 * 128], BF16, tag="e")
        nc.scalar.activation(
            e_sb[:, 0:512], sc_a[:],
            mybir.ActivationFunctionType.Exp, scale=SCALE,
        )
        nc.scalar.activation(
            e_sb[:, 512:1024], sc_b[:],
            mybir.ActivationFunctionType.Exp, scale=SCALE,
        )
        return e_sb

    def denom_half(e_sb):
        # denominators: block-identity matmul sums over tk (partition
        # axis) AND broadcasts the result to all 64 partitions of the
        # matching chunk. Garbage cross-chunk scores are excluded by the
        # zeros in the stationary.
        e3 = e_sb[:].rearrange("p (h q) -> p h q", h=8)
        sm_ps = ps_sm.tile([P, 512], F32, tag="sm")
        nc.tensor.matmul(
            sm_ps[0:64, :],
            lhsT=consts_sb[:, 0:64],
            rhs=e3[:, :, 0:64],
            start=True, stop=True,
        )
        nc.tensor.matmul(
            sm_ps[64:128, :],
            lhsT=consts_sb[:, 64:128],
            rhs=e3[:, :, 64:128],
            start=True, stop=True,
        )
        rb = rbpool.tile([P, 512], F32, tag="rb")
        nc.vector.reciprocal_approx_fast(out=rb[:], in_=sm_ps[:])
        # normalize: En = E * rb  (valid block-diagonal regions only)
        en = en_tiles[half_idx[0] % 4]
        half_idx[0] += 1
        en3 = en[:].rearrange("p (h q) -> p h q", h=8)
        rb3 = rb[:].rearrange("p (h q) -> p h q", h=8)
        nc.vector.tensor_mul(
            en3[0:64, :, 0:64], e3[0:64, :, 0:64], rb3[0:64, :, :]
        )
        nc.vector.tensor_mul(
            en3[64:128, :, 64:128], e3[64:128, :, 64:128], rb3[64:128, :, :]
        )
        return en

    def attnv_half(v_sb, en, attn_sb, st, hh, at_pool, at_tag):
        # attn @ v : outT[d(head), tok] — full-partition contract, En's
        # zeros kill the cross-chunk terms. JORDER alternates the output
        # column group (hb) for sub-array overlap.
        at_ps = at_pool.tile([P, 512], F32, tag=at_tag, name="at_ps")
        for j in JORDER:
            h = hh * 8 + 2 * (j % 4) + (j // 4)
            hb = (h % 2) * 64
            nc.tensor.matmul(
                at_ps[hb:hb + 64, (j % 4) * 128:(j % 4 + 1) * 128],
                lhsT=v_sb[:, h * 64:(h + 1) * 64],
                rhs=en[:, j * 128:(j + 1) * 128],
                start=True, stop=True,
            )
        nc.scalar.copy(
            attn_sb[:, st * 1024 + hh * 512: st * 1024 + (hh + 1) * 512],
            at_ps[:],
        )

    for g in range(G):
        x_t = x_tiles[g]

        # ---- q/k projections: out layout [feat, tok] -------------------
        q_sb = qkpool.tile([P, 8 * GT], BF16, tag="q")
        k_sb = qkpool.tile([P, 8 * GT], BF16, tag="k")
        attn_sb = apool.tile([P, NST * 8 * 128], BF16, tag="attn",
                             name=f"attn_{g}")
        for fc in range(16):
            qk_ps = ps_big.tile([P, GT], F32, tag="big")
            for kc in range(KC):
                nc.tensor.matmul(
                    qk_ps[:],
                    lhsT=wqk_sb[:, fc * 1024 + kc * 128: fc * 1024 + kc * 128 + 128],
                    rhs=x_t[:, kc * GT:(kc + 1) * GT],
                    start=(kc == 0),
                    stop=(kc == KC - 1),
                )
            dst = q_sb if fc < 8 else k_sb
            sl = dst[:, (fc % 8) * GT:(fc % 8 + 1) * GT]
            if fc % 2 == 0:
                nc.vector.tensor_copy(sl, qk_ps[:])
            else:
                nc.scalar.copy(sl, qk_ps[:])

        # prefetch next group's x now, before this group's y store can
        # block the Sync queue
        if g + 1 < G:
            x_n = xpool.tile([P, KC * GT], BF16, tag="x")
            load_x(x_n, (g + 1) * GT)
            x_tiles.append(x_n)
        if g == 0:
            load_wout()

        # ---- per 128-token subtile: v projection + attention -----------
        for st in range(NST):
            fill = make_v(g, st)
            fill(0, 4)
            v_sb = fill(4, KC)
            e0 = scores_half(q_sb, k_sb, st, 0)
            en0 = denom_half(e0)
            attnv_half(v_sb, en0, attn_sb, st, 0, ps_at, "at")
            e1 = scores_half(q_sb, k_sb, st, 1)
            en1 = denom_half(e1)
            attnv_half(v_sb, en1, attn_sb, st, 1, ps_at, "at")

        # ---- final projection ------------------------------------------
        # For the last group there is no following work to hide the
        # attention→final serialization, so split it into two token halves:
        # the first half only needs subtiles 0..NST/2-1 and overlaps the
        # rest of the attention. y is packed ec-major into one tile so the
        # store is a single DMA.
        a3 = attn_sb[:].rearrange("p (s h t) -> p s h t", s=NST, h=8)
        halves = ((0, NST // 2), (NST // 2, NST)) if g == G - 1 \
            else ((0, NST),)
        for s0, s1 in halves:
            ht = (s1 - s0) * ST
            y_sb = ypool.tile([P, 8 * GT], BF16, tag="y")
            for ec in range(8):
                f_ps = ps_big.tile([P, GT], F32, tag="big")
                for hp in range(KC):
                    nc.tensor.matmul(
                        f_ps[:, 0:ht],
                        lhsT=wout_sb[:, hp * 1024 + ec * 128: hp * 1024 + ec * 128 + 128],
                        rhs=a3[:, s0:s1, hp, :],
                        start=(hp == 0),
                        stop=(hp == KC - 1),
                    )
                sl = y_sb[:, ec * ht:(ec + 1) * ht]
                if ec % 2 == 0:
                    nc.vector.tensor_copy(sl, f_ps[:, 0:ht])
                else:
                    nc.scalar.copy(sl, f_ps[:, 0:ht])
            if g == G - 1 and s0 > 0:
                # final store: split so the first half streams while the
                # last PSUM evacuations finish
                nc.sync.dma_start(
                    yT[:, 0:4, g * GT + s0 * ST: g * GT + s1 * ST],
                    y_sb[:, 0:4 * ht].rearrange("p (e t) -> p e t", e=4),
                )
                nc.sync.dma_start(
                    yT[:, 4:8, g * GT + s0 * ST: g * GT + s1 * ST],
                    y_sb[:, 4 * ht:8 * ht].rearrange("p (e t) -> p e t", e=4),
                )
            else:
                nc.sync.dma_start(
                    yT[:, :, g * GT + s0 * ST: g * GT + s1 * ST],
                    y_sb[:, 0:8 * ht].rearrange("p (e t) -> p e t", e=8),
                )

    ctx.close()


def build_nc(T):
    nc = bacc.Bacc("TRN2", target_bir_lowering=False, debug=False)
    xT = nc.dram_tensor("xT", [P, KC, T], BF16, kind="ExternalInput").ap()
    wqkvT = nc.dram_tensor("wqkvT", [P, NFC, 1024], BF16, kind="ExternalInput").ap()
    woutT = nc.dram_tensor("woutT", [P, KC, 1024], BF16, kind="ExternalInput").ap()
    consts = nc.dram_tensor("consts", [P, 128], BF16, kind="ExternalInput").ap()
    yT = nc.dram_tensor("yT", [P, KC, T], BF16, kind="ExternalOutput").ap()
    with tile.TileContext(nc) as tc:
        build_body(tc, yT, xT, wqkvT, woutT, consts, T)
    nc.compile()
    return nc


def make_consts():
    c = np.zeros((P, 128), dtype=BF16_NP)
    c[0:64, 0:64] = 1
    c[64:128, 64:128] = 1
    return c


def prep_inputs(x, w_qkv, w_out, T):
    """Host-side shard + transpose + cast. Returns in_maps list for SPMD."""
    tok = x.shape[0] * x.shape[1]
    flat = np.ascontiguousarray(x.reshape(tok, DIM))
    # [fc, fr, kc, p] -> [p, fc, kc, fr] -> [p, fc, kc*128+fr]
    wqkvT = np.ascontiguousarray(
        w_qkv.reshape(NFC, 128, KC, 128).transpose(3, 0, 2, 1).reshape(P, NFC, 1024)
    ).astype(BF16_NP)
    woutT = np.ascontiguousarray(
        w_out.T.reshape(KC, P, 1024).transpose(1, 0, 2)
    ).astype(BF16_NP)
    consts = make_consts()
    n_cores = tok // T
    in_maps = []
    for c in range(n_cores):
        shard = flat[c * T:(c + 1) * T]           # [T, 1024]
        xTc = np.ascontiguousarray(
            shard.T.reshape(KC, P, T).transpose(1, 0, 2)
        ).astype(BF16_NP)
        in_maps.append({"xT": xTc, "wqkvT": wqkvT, "woutT": woutT,
                        "consts": consts})
    return in_maps


def postprocess(results, b_out, bshape, T):
    outs = []
    for r in results:
        yT = np.asarray(r["yT"], dtype=np.float32)    # [128, 8, T] (bf16)
        outs.append(yT.transpose(2, 1, 0).reshape(T, DIM))   # [T, 1024]
    y = np.concatenate(outs, axis=0)                  # [tok, 1024]
    y = y + np.asarray(b_out, dtype=np.float32)[None, :]
    return y.reshape(*bshape, DIM)


_CACHED = {}


def kernel(x, w_qkv, w_out, b_out):
    from concourse.bass_utils import run_bass_kernel_spmd

    x = np.asarray(x)
    b, n, _ = x.shape
    T = (b * n) // N_CORES
    if T not in _CACHED:
        _CACHED[T] = build_nc(T)
    nc = _CACHED[T]
    in_maps = prep_inputs(x, np.asarray(w_qkv), np.asarray(w_out), T)
    res = run_bass_kernel_spmd(nc, in_maps, list(range(N_CORES)))
    return postprocess(res.results, b_out, (b, n), T)


if __name__ == "__main__":
    nc = build_nc(2048)
    print("built ok")
